# revision 27
# baseline (speedup 1.0000x reference)
"""CPD reconstruction at observed entries (embedding-lookup style) on 8 TRN2 cores.

rec[n] = sum_r f0[i0[n],r] * f1[i1[n],r] * f2[i2[n],r]   for n in [0, 1M)

Strategy (per sharding hint): data-parallel over the nnz axis across the 8
cores; the factor matrices are replicated to every core's HBM as one
concatenated table F = [f0; f1; f2] (300000 x 32 f32).  Each core turns its
125k x 3 indices into row offsets into F (idx + mode*100000, one DVE add),
gathers all three modes' rows with one indirect DMA per tile (one 128B
descriptor per row), multiplies the three gathered rows elementwise on DVE
and reduces over rank, then writes its 125k f32 results back with one
contiguous DMA.
"""

import numpy as np

NNZ = 1_000_000
RANK = 32
ROWS = 100_000
N_CORES = 8
N_PER_CORE = NNZ // N_CORES  # 125_000
P = 128
W = -(-N_PER_CORE // P)  # 977 entries per partition
N_PAD = P * W  # 125_056
TILE_K = 64  # entries per partition per tile

_cache: dict = {}


def _build(nwords: int, split_waits: bool = True):
    """Build the per-core Bass program.

    nwords: int32 words per index entry in DRAM (6 when the host hands us an
    int64 [N,3] array viewed as int32, 3 for a native int32 [N,3] array).
    """
    import concourse.bass as bass
    import concourse.mybir as mybir
    from concourse.tile import TileContext

    stride = nwords // 3  # int32 words between mode-m and mode-m+1 of an entry

    nc = bass.Bass()
    # Last 3 columns of idx32 hold the per-mode row-offset bias
    # [0, ROWS, 2*ROWS] so one DMA brings in indices and bias together.
    idx32 = nc.dram_tensor(
        "idx32", [P, W * nwords + 3], mybir.dt.int32, kind="ExternalInput"
    )
    ftab = nc.dram_tensor(
        "ftab", [3 * ROWS, RANK], mybir.dt.float32, kind="ExternalInput"
    )
    out = nc.dram_tensor("out", [P, W], mybir.dt.float32, kind="ExternalOutput")

    with TileContext(nc) as tc:
        with (
            tc.tile_pool(name="io", bufs=1) as io_pool,
            tc.tile_pool(name="gat", bufs=6) as gat_pool,
            tc.tile_pool(name="prd", bufs=3) as prd_pool,
        ):
            out_sb = io_pool.tile([P, W], mybir.dt.float32)
            # Resident copy of all this core's indices + bias tail
            # (one big HWDGE DMA).
            idx_sb = io_pool.tile([P, W * nwords + 3], mybir.dt.int32)
            nc.sync.dma_start(out=idx_sb[:], in_=idx32[:])
            bias_sb = idx_sb[:, W * nwords : W * nwords + 3]
            # offs[p, 3j+m] = idx[entry p*W+j, mode m] + m*ROWS
            offs = io_pool.tile([P, 3 * W], mybir.dt.int32)
            src = idx_sb[:, : W * nwords].rearrange("p (j s) -> p j s", s=nwords)
            if stride == 2:
                src = src[:, :, 0::2]
            nc.vector.tensor_add(
                out=offs[:].rearrange("p (j m) -> p j m", m=3),
                in0=src,
                in1=bias_sb[:, None, :].to_broadcast([P, W, 3]),
            )
            # The HW indirect DMA consumes ONE offset per partition (first
            # element of that partition's offset row), so each gather moves
            # 128 single rows.  Group CH entry-columns per compute step so
            # DVE work and cross-engine sync points amortize over 3*CH
            # gathers instead of 3.
            CH = 16
            j0 = 0
            while j0 < W:
                C = min(CH, W - j0)
                g = gat_pool.tile([P, CH * 3 * RANK], mybir.dt.float32, tag="g")
                for cc in range(C):
                    for m in range(3):
                        nc.gpsimd.indirect_dma_start(
                            out=g[
                                :,
                                (cc * 3 + m) * RANK : (cc * 3 + m + 1) * RANK,
                            ],
                            out_offset=None,
                            in_=ftab[:],
                            in_offset=bass.IndirectOffsetOnAxis(
                                ap=offs[
                                    :, 3 * (j0 + cc) + m : 3 * (j0 + cc) + m + 1
                                ],
                                axis=0,
                            ),
                        )
                v = g[:, : C * 3 * RANK].rearrange(
                    "p (c m r) -> p c m r", m=3, r=RANK
                )
                tmp = prd_pool.tile([P, CH * RANK], mybir.dt.float32, tag="tmp")
                tv = tmp[:, : C * RANK].rearrange("p (c r) -> p c r", r=RANK)
                nc.vector.tensor_mul(out=tv, in0=v[:, :, 0, :], in1=v[:, :, 1, :])
                nc.vector.tensor_mul(out=tv, in0=tv, in1=v[:, :, 2, :])
                nc.vector.reduce_sum(
                    out=out_sb[:, j0 : j0 + C],
                    in_=tv,
                    axis=mybir.AxisListType.X,
                )
                j0 += C
            nc.sync.dma_start(out=out[:], in_=out_sb[:])

    if split_waits:
        _split_multi_waits(nc, mybir)
    return nc


def _split_multi_waits(nc, mybir):
    """The TRN2 ISA embeds at most ONE sem wait per instruction; Tile
    sometimes attaches several.  Hoist the extras into standalone
    EventSemaphore instructions placed immediately before the owner in the
    same block — same engine queue, same order, identical semantics."""
    for blk in nc.m.functions[0].blocks:
        new_insts = []
        for inst in blk.instructions:
            si = inst.sync_info
            if (
                type(inst).__name__ == "InstDMACopy"
                and inst.engine == mybir.EngineType.Pool
                and si is not None
                and si.on_wait
                and len(si.on_wait) == 2
            ):
                # Gather with [DVE WAR, DMASW WAW]: the DMASW wait is
                # transitively implied by the DVE one (the DVE readers of
                # this slot already waited on the prior gather's completion
                # sem, and DVE is in-order) — drop it instead of splitting,
                # keeping the Pool queue free of extra wait instructions.
                names = [(w.ant_name or "") for w in si.on_wait]
                if any(n.startswith("DVE") for n in names) and any(
                    n.startswith("DMASW") for n in names
                ):
                    si.on_wait = [
                        w
                        for w in si.on_wait
                        if (w.ant_name or "").startswith("DVE")
                    ]
                    new_insts.append(inst)
                    continue
            if si is not None and si.on_wait and len(si.on_wait) > 1:
                extra, keep = list(si.on_wait[:-1]), [si.on_wait[-1]]
                for j, w in enumerate(extra):
                    new_insts.append(
                        mybir.InstEventSemaphore(
                            name=f"{inst.name}-esw{j}",
                            engine=inst.engine,
                            ins=[],
                            outs=[],
                            sync_info=mybir.SyncInfo(on_wait=[w], on_update=[]),
                        )
                    )
                si.on_wait = keep
            new_insts.append(inst)
        blk.instructions = new_insts


def _get_nc(nwords: int):
    if nwords not in _cache:
        _cache[nwords] = _build(nwords)
    return _cache[nwords]


def _prep_in_maps(idxs, f0, f1, f2):
    idxs = np.asarray(idxs)
    f0 = np.asarray(f0, dtype=np.float32)
    f1 = np.asarray(f1, dtype=np.float32)
    f2 = np.asarray(f2, dtype=np.float32)
    ftab = np.ascontiguousarray(np.concatenate([f0, f1, f2], axis=0))
    bias3 = np.array([0, ROWS, 2 * ROWS], dtype=np.int32)

    if idxs.dtype == np.int64:
        idx32 = np.ascontiguousarray(idxs).view(np.int32)  # [NNZ, 6], low word first
        nwords = 6
    elif idxs.dtype == np.int32:
        idx32 = np.ascontiguousarray(idxs)  # [NNZ, 3]
        nwords = 3
    else:
        raise ValueError(f"unsupported idxs dtype {idxs.dtype}")

    in_maps = []
    for c in range(N_CORES):
        sl = idx32[c * N_PER_CORE : (c + 1) * N_PER_CORE]
        padded = np.zeros((N_PAD, nwords), dtype=np.int32)
        padded[:N_PER_CORE] = sl
        arr = np.empty((P, W * nwords + 3), dtype=np.int32)
        arr[:, : W * nwords] = padded.reshape(P, W * nwords)
        arr[:, W * nwords :] = bias3
        in_maps.append({"idx32": arr, "ftab": ftab})
    return in_maps, nwords


def run(inputs: dict, trace: bool = False):
    """Run the kernel on 8 cores; returns (full_output, BassKernelResults)."""
    from concourse.bass_utils import run_bass_kernel_spmd

    in_maps, nwords = _prep_in_maps(
        inputs["idxs"], inputs["f0"], inputs["f1"], inputs["f2"]
    )
    nc = _get_nc(nwords)
    res = run_bass_kernel_spmd(
        nc,
        in_maps,
        core_ids=list(range(N_CORES)),
        trace=trace,
    )
    out = np.concatenate(
        [r["out"].reshape(-1)[:N_PER_CORE] for r in res.results]
    )
    return out, res


def kernel(**inputs) -> np.ndarray:
    out, _ = run(inputs, trace=False)
    return out
